# revision 19
# baseline (speedup 1.0000x reference)
import sys, os
sys.path.insert(0, "/opt/trn_rl_repo")
import numpy as np
import ml_dtypes
from contextlib import ExitStack

import concourse.bass as bass
import concourse.tile as tile
from concourse import bacc, mybir
from concourse.bass_utils import run_bass_kernel_spmd

BF16 = ml_dtypes.bfloat16
F32 = np.float32

B, C, T = 4, 512, 16384
DILATIONS = (1, 2, 4)
SLOPE = 0.1
NCORES = 8
HALF = T // 2            # 8192 per core
H = 64                   # halo per side (max reach used: 3+1 + 6+1 + 12+1 = 24)
SL = HALF + 2 * H        # 8320 slice length
TC = 2048                # chunk kept size
E = TC + 2 * H           # 2176 chunk buffer extent
NCHUNK = HALF // TC      # 4
NT = E // 128            # 17 t-tiles per chunk buffer
K3WIN = [(1, 435), (436, 435), (871, 435), (1306, 435), (1741, 434)]
TAUA = [(t_, a_) for t_ in range(3) for a_ in range(4)]
EQ = E // 4              # 544, quarter split for x DMA / prelu pipelining

_NC = None


def _build(with_bias=True):
    nc = bacc.Bacc("TRN2", target_bir_lowering=False, debug=False, num_devices=NCORES)
    dt = mybir.dt

    xs = nc.dram_tensor("xs", [C, SL], dt.bfloat16, kind="ExternalInput").ap()
    mks = nc.dram_tensor("mks", [12 * 128, SL], dt.bfloat16, kind="ExternalInput").ap()
    em = nc.dram_tensor("em", [1, 2 * NCHUNK], dt.float32, kind="ExternalInput").ap()
    w1 = nc.dram_tensor("w1", [128, 36, 512], dt.bfloat16, kind="ExternalInput").ap()
    wa = nc.dram_tensor("wa", [128, 36, 512], dt.bfloat16, kind="ExternalInput").ap()
    bsum = nc.dram_tensor("bsum", [1, 3 * 512], dt.bfloat16, kind="ExternalInput").ap()
    ba_r = nc.dram_tensor("ba_r", [1, 3 * 512], dt.bfloat16, kind="ExternalInput").ap()
    out = nc.dram_tensor("out", [C, HALF], dt.bfloat16, kind="ExternalOutput").ap()

    with tile.TileContext(nc) as tc:
        with ExitStack() as ctx:
            consts = ctx.enter_context(tc.tile_pool(name="consts", bufs=1))
            xpool = ctx.enter_context(tc.tile_pool(name="xpool", bufs=3))
            xtpool = ctx.enter_context(tc.tile_pool(name="xtpool", bufs=1))
            mpool = ctx.enter_context(tc.tile_pool(name="mpool", bufs=4))
            wapool = ctx.enter_context(tc.tile_pool(name="wapool", bufs=2))
            ypool = ctx.enter_context(tc.tile_pool(name="ypool", bufs=8))
            ztpool = ctx.enter_context(tc.tile_pool(name="ztpool", bufs=3))
            eypool = ctx.enter_context(tc.tile_pool(name="eypool", bufs=4))
            zpool = ctx.enter_context(tc.tile_pool(name="zpool", bufs=2))
            psA = ctx.enter_context(tc.tile_pool(name="psA", bufs=3, space="PSUM"))
            psB = ctx.enter_context(tc.tile_pool(name="psB", bufs=5, space="PSUM"))

            # ---- constants (w1 split per conv so layer-0 work starts early;
            # weights go on the scalar queue, x on sync, masks on gpsimd so the
            # critical head transfers run in parallel) ----
            w1_sb = consts.tile([128, 36, 512], dt.bfloat16)
            # layer 0 pieces first (yf conv first: it is the first matmul issued);
            # l1/l2 pieces are emitted later on the gpsimd queue behind l0 masks.
            for cv in (2, 1, 0):
                i0 = cv * 4
                nc.scalar.dma_start(out=w1_sb[:, i0 : i0 + 4, :], in_=w1[:, i0 : i0 + 4, :])
            bsum_sb = consts.tile([1, 3 * 512], dt.bfloat16)
            nc.gpsimd.dma_start(out=bsum_sb[:], in_=bsum)
            ba_sb = consts.tile([1, 3 * 512], dt.bfloat16)
            nc.gpsimd.dma_start(out=ba_sb[:], in_=ba_r)
            em_sb = consts.tile([128, 2 * NCHUNK], dt.float32)
            nc.gpsimd.dma_start(
                out=em_sb[:],
                in_=bass.AP(tensor=em.tensor, offset=0, ap=[[0, 128], [1, 2 * NCHUNK]]),
            )
            ones128 = consts.tile([1, 128], dt.bfloat16)
            nc.vector.memset(ones128[:], 1.0)
            ones512 = consts.tile([1, 512], dt.bfloat16)
            nc.vector.memset(ones512[:], 1.0)

            for ck in range(NCHUNK):
                cb = ck * TC

                x_cur = xpool.tile([128, 4, E], dt.bfloat16, tag="x")
                xt = xtpool.tile([128, 4, E], dt.bfloat16, tag="xt")
                npiece = 8 if ck == 0 else 4
                pw = E // npiece
                for q in range(npiece):
                    c0 = q * pw
                    nc.sync.dma_start(
                        out=x_cur[:, :, c0 : c0 + pw],
                        in_=xs[:, cb + c0 : cb + c0 + pw].rearrange("(a p) t -> p a t", p=128),
                    )
                    nc.scalar.activation(
                        out=xt[:, :, c0 : c0 + pw], in_=x_cur[:, :, c0 : c0 + pw],
                        func=mybir.ActivationFunctionType.Prelu, alpha=SLOPE,
                    )

                for l in range(3):
                    masks = {}
                    for i, nm in enumerate(("P_diag", "F_diag", "EDGE")):
                        mk = mpool.tile([128, E], dt.bfloat16, tag="mask")
                        r0 = (l * 4 + (0, 2, 1)[i]) * 128
                        nc.gpsimd.dma_start(out=mk[:, 0 : E // 2], in_=mks[r0 : r0 + 128, cb : cb + E // 2])
                        nc.gpsimd.dma_start(out=mk[:, E // 2 : E], in_=mks[r0 : r0 + 128, cb + E // 2 : cb + E])
                        masks[nm] = mk
                    wa_sb = wapool.tile([128, 12, 512], dt.bfloat16, tag="wa")
                    nc.sync.dma_start(out=wa_sb[:], in_=wa[:, l * 12 : (l + 1) * 12, :])
                    if ck == 0 and l == 0:
                        # deferred: layer-1/2 1x1 weights, behind layer-0 masks
                        for ll in (1, 2):
                            for cv in (2, 1, 0):
                                i0 = (ll * 3 + cv) * 4
                                nc.gpsimd.dma_start(out=w1_sb[:, i0 : i0 + 4, :], in_=w1[:, i0 : i0 + 4, :])

                    ycT = [None] * NT
                    ypS = [None] * NT
                    yfS = [None] * NT
                    zTs = [None] * NT

                    def conv_step(j):
                        yc = psA.tile([128, 512], dt.float32, tag="ycT")
                        yp = psB.tile([128, 512], dt.float32, tag="ps")
                        yf = psB.tile([128, 512], dt.float32, tag="ps")
                        for a in range(4):
                            lhs = xt[:, a, 128 * j : 128 * (j + 1)]
                            st = a == 0
                            nc.tensor.matmul(yf[:], lhs, w1_sb[:, (l * 3 + 2) * 4 + a, :], start=st, stop=a == 3)
                            nc.tensor.matmul(yp[:], lhs, w1_sb[:, (l * 3 + 1) * 4 + a, :], start=st, stop=a == 3)
                            nc.tensor.matmul(yc[:], lhs, w1_sb[:, (l * 3 + 0) * 4 + a, :], start=st, stop=False)
                        if with_bias:
                            nc.tensor.matmul(yc[:], ones128[:], bsum_sb[:, l * 512 : (l + 1) * 512], start=False, stop=False)
                        ycT[j] = yc
                        yp_s = ypool.tile([128, 512], dt.bfloat16, tag="yps")
                        nc.vector.tensor_copy(out=yp_s[:], in_=yp[:])
                        yf_s = ypool.tile([128, 512], dt.bfloat16, tag="yps")
                        nc.scalar.activation(out=yf_s[:], in_=yf[:], func=mybir.ActivationFunctionType.Copy)
                        ypS[j] = yp_s
                        yfS[j] = yf_s

                    eys = [None] * NT

                    def gather_diag(j):
                        tj = slice(128 * j, 128 * (j + 1))
                        nc.tensor.matmul(ycT[j][:], masks["P_diag"][:, tj], ypS[j][:], start=False, stop=False)
                        nc.tensor.matmul(ycT[j][:], masks["F_diag"][:, tj], yfS[j][:], start=False, stop=False)

                    def edge_dma(j):
                        # edge tile: rows 0:64 = tail of ypS[j-1], rows 64:128 = head of
                        # yfS[j+1]; one combined mask matmul replaces P_sub + F_sup.
                        ey = eypool.tile([128, 512], dt.bfloat16, tag="ey")
                        if j > 0:
                            nc.sync.dma_start(out=ey[0:64, :], in_=ypS[j - 1][64:128, :])
                        else:
                            nc.vector.memset(ey[0:64, :], 0.0)
                        if j < NT - 1:
                            nc.sync.dma_start(out=ey[64:128, :], in_=yfS[j + 1][0:64, :])
                        else:
                            nc.vector.memset(ey[64:128, :], 0.0)
                        eys[j] = ey

                    def gather_edge(j):
                        tj = slice(128 * j, 128 * (j + 1))
                        nc.tensor.matmul(ycT[j][:], masks["EDGE"][:, tj], eys[j][:], start=False, stop=True)
                        zt = ztpool.tile([128, 512], dt.bfloat16, tag="zt")
                        nc.scalar.activation(out=zt[:], in_=ycT[j][:], func=mybir.ActivationFunctionType.Prelu, alpha=SLOPE)
                        zTs[j] = zt

                    z = zpool.tile([128, 4, E], dt.bfloat16, tag="z")

                    def transpose_step(j):
                        eng = nc.sync if j % 2 == 0 else nc.scalar
                        eng.dma_start_transpose(out=z[:, :, 128 * j : 128 * (j + 1)], in_=zTs[j][:])
                        if j == 0:
                            for m in range(4):
                                nc.vector.tensor_scalar(
                                    out=z[:, m, H - 1 : H], in0=z[:, m, H - 1 : H],
                                    scalar1=em_sb[:, 2 * ck : 2 * ck + 1], scalar2=None,
                                    op0=mybir.AluOpType.mult,
                                )
                        if j == NT - 1:
                            for m in range(4):
                                nc.vector.tensor_scalar(
                                    out=z[:, m, H + TC : H + TC + 1], in0=z[:, m, H + TC : H + TC + 1],
                                    scalar1=em_sb[:, 2 * ck + 1 : 2 * ck + 2], scalar2=None,
                                    op0=mybir.AluOpType.mult,
                                )

                    for j in range(NT):
                        conv_step(j)
                        if j >= 1:
                            edge_dma(j - 1)
                            gather_diag(j - 1)
                        if j >= 2:
                            gather_edge(j - 2)
                            transpose_step(j - 2)
                    edge_dma(NT - 1)
                    gather_diag(NT - 1)
                    gather_edge(NT - 2)
                    transpose_step(NT - 2)
                    gather_edge(NT - 1)
                    transpose_step(NT - 1)

                    # ---- k=3 conv + residual, window-outer so early windows
                    # overlap the tail of the conv/gather/transpose phase ----
                    x_next = xpool.tile([128, 4, E], dt.bfloat16, tag="x")
                    last = l == 2
                    if not last:
                        xt_n = xtpool.tile([128, 4, E], dt.bfloat16, tag="xt")
                        for m in range(4):
                            nc.vector.memset(xt_n[:, m, 0:1], 0.0)
                            nc.vector.memset(xt_n[:, m, E - 1 : E], 0.0)
                    for wi, (w0, wn) in enumerate(K3WIN):
                        for m in range(4):
                            pk = psB.tile([128, 512], dt.float32, tag="ps")
                            for ki, (tau, a) in enumerate(TAUA):
                                lhs = wa_sb[:, tau * 4 + a, 128 * m : 128 * (m + 1)]
                                rhs = z[:, a, w0 + tau - 1 : w0 + tau - 1 + wn]
                                nc.tensor.matmul(pk[:, 0:wn], lhs, rhs, start=ki == 0,
                                                 stop=(ki == 11) and not with_bias)
                            if with_bias:
                                nc.tensor.matmul(
                                    pk[:, 0:wn], ba_sb[:, l * 512 + 128 * m : l * 512 + 128 * (m + 1)],
                                    ones512[:, 0:wn], start=False, stop=True,
                                )
                            nc.vector.tensor_tensor(
                                out=x_next[:, m, w0 : w0 + wn], in0=pk[:, 0:wn],
                                in1=x_cur[:, m, w0 : w0 + wn], op=mybir.AluOpType.add,
                            )
                            if not last:
                                nc.scalar.activation(
                                    out=xt_n[:, m, w0 : w0 + wn], in_=x_next[:, m, w0 : w0 + wn],
                                    func=mybir.ActivationFunctionType.Prelu, alpha=SLOPE,
                                )
                        if last:
                            lo, hi = max(w0, H), min(w0 + wn, H + TC)
                            if hi > lo:
                                nc.sync.dma_start(
                                    out=out[:, cb + lo - H : cb + hi - H].rearrange("(a p) t -> p a t", p=128),
                                    in_=x_next[:, :, lo:hi],
                                )
                    x_cur = x_next
                    if not last:
                        xt = xt_n

    nc.compile()
    return nc


def _host_inputs(x, d, Wc, bc, Wp, bp, Wf, bf, Wa, ba):
    x = np.asarray(x, dtype=F32)
    d = np.asarray(d, dtype=F32)
    Wc, Wp, Wf = (np.asarray(w, dtype=F32) for w in (Wc, Wp, Wf))
    Wa = np.asarray(Wa, dtype=F32)
    bc, bp, bf, ba = (np.asarray(v, dtype=F32) for v in (bc, bp, bf, ba))

    w1 = np.empty((128, 36, 512), dtype=BF16)
    wa = np.empty((128, 36, 512), dtype=BF16)
    for l in range(3):
        for cv, W in enumerate((Wc, Wp, Wf)):
            wt = W[l].T.astype(BF16)
            for a in range(4):
                w1[:, (l * 3 + cv) * 4 + a, :] = wt[a * 128 : (a + 1) * 128, :]
        for tau in range(3):
            wt = Wa[l][:, :, tau].T.astype(BF16)
            for a in range(4):
                wa[:, (l * 3 + tau) * 4 + a, :] = wt[a * 128 : (a + 1) * 128, :]
    bsum = (bc + bp + bf).reshape(1, -1).astype(BF16)
    ba_r = ba.reshape(1, -1).astype(BF16)

    p_ar = np.arange(128, dtype=np.int64)[:, None]
    tilebase = 128 * (np.arange(SL, dtype=np.int64) // 128)[None, :]

    in_maps = []
    for core in range(NCORES):
        b, h = core // 2, core % 2
        g0 = h * HALF
        lo = g0 - H
        xsl = np.zeros((C, SL), dtype=BF16)
        dsl = np.zeros((1, SL), dtype=F32)
        s0 = max(0, lo)
        s1 = min(T, g0 + HALF + H)
        xsl[:, s0 - lo : s1 - lo] = x[b, :, s0:s1].astype(BF16)
        dsl[:, s0 - lo : s1 - lo] = d[b, :, s0:s1]

        tg = (np.arange(SL, dtype=np.float64) + lo).astype(F32)
        cl = float(max(0, lo))
        chq = float(min(T - 1, g0 + HALF + H - 1))
        mks = np.zeros((12 * 128, SL), dtype=BF16)
        for l in range(3):
            dil = np.float32(DILATIONS[l])
            dila = (dsl[0] * dil).astype(F32)
            for gi, sgn in ((0, np.float32(-1.0)), (1, np.float32(1.0))):
                u = (tg + sgn * dila).astype(F32)
                idxg = np.clip(np.round(u), cl, chq).astype(np.int64)
                rel = (idxg - lo)[None, :] - tilebase
                diag = (rel == p_ar).astype(BF16)
                i_diag = l * 4 + (0 if gi == 0 else 2)
                mks[i_diag * 128 : (i_diag + 1) * 128, :] = diag
                # combined edge plane at slot l*4+1:
                # rows 0:64  <- P_sub rows 64:128 (sources = tail of prev tile)
                # rows 64:128 <- F_sup rows 0:64  (sources = head of next tile)
                e0 = (l * 4 + 1) * 128
                if gi == 0:
                    mks[e0 : e0 + 64, :] = (rel == (p_ar[64:128] - 128)).astype(BF16)
                else:
                    mks[e0 + 64 : e0 + 128, :] = (rel == (p_ar[0:64] + 128)).astype(BF16)

        em = np.ones((1, 2 * NCHUNK), dtype=F32)
        if h == 0:
            em[0, 0] = 0.0
        if h == 1:
            em[0, 2 * NCHUNK - 1] = 0.0
        in_maps.append(
            dict(xs=xsl, mks=mks, em=em, w1=w1, wa=wa, bsum=bsum, ba_r=ba_r)
        )
    return in_maps


_NC_BIAS = None


def kernel(**inputs):
    global _NC, _NC_BIAS
    wb = any(np.any(np.asarray(inputs[k])) for k in ("bc", "bp", "bf", "ba"))
    if _NC is None or _NC_BIAS != wb:
        _NC = _build(with_bias=wb)
        _NC_BIAS = wb
    in_maps = _host_inputs(**inputs)
    res = run_bass_kernel_spmd(_NC, in_maps, core_ids=list(range(NCORES)), trace=False)
    out = np.empty((B, C, T), dtype=F32)
    for core in range(NCORES):
        b, h = core // 2, core % 2
        out[b, :, h * HALF : (h + 1) * HALF] = np.asarray(res.results[core]["out"]).astype(F32)
    return out


# revision 23
# speedup vs baseline: 1.2783x; 1.2783x over previous
import sys, os
sys.path.insert(0, "/opt/trn_rl_repo")
import numpy as np
import ml_dtypes
from contextlib import ExitStack

import concourse.bass as bass
import concourse.tile as tile
from concourse import bacc, mybir
from concourse.bass_utils import run_bass_kernel_spmd

BF16 = ml_dtypes.bfloat16
F32 = np.float32

B, C, T = 4, 512, 16384
DILATIONS = (1, 2, 4)
SLOPE = 0.1
NCORES = 8
HALF = T // 2            # 8192 per core
H = 64                   # halo per side (max reach used: 3+1 + 6+1 + 12+1 = 24)
SL = HALF + 2 * H        # 8320 slice length
TC = 2048                # chunk kept size
E = TC + 2 * H           # 2176 chunk buffer extent
NCHUNK = HALF // TC      # 4
NT = E // 128            # 17 t-tiles per chunk buffer
K3WIN = [(1, 435), (436, 435), (871, 435), (1306, 435), (1741, 434)]
TAUA = [(t_, a_) for t_ in range(3) for a_ in range(4)]
EQ = E // 4              # 544, quarter split for x DMA / prelu pipelining

_NC = None


def _build(with_bias=True):
    nc = bacc.Bacc("TRN2", target_bir_lowering=False, debug=False, num_devices=NCORES)
    dt = mybir.dt

    xs = nc.dram_tensor("xs", [C, SL], dt.bfloat16, kind="ExternalInput").ap()
    mks = nc.dram_tensor("mks", [12 * 128, SL], dt.bfloat16, kind="ExternalInput").ap()
    em = nc.dram_tensor("em", [1, 2 * NCHUNK], dt.float32, kind="ExternalInput").ap()
    w1 = nc.dram_tensor("w1", [128, 36, 512], dt.bfloat16, kind="ExternalInput").ap()
    wa = nc.dram_tensor("wa", [128, 36, 512], dt.bfloat16, kind="ExternalInput").ap()
    bsum = nc.dram_tensor("bsum", [1, 3 * 512], dt.bfloat16, kind="ExternalInput").ap()
    ba_r = nc.dram_tensor("ba_r", [1, 3 * 512], dt.bfloat16, kind="ExternalInput").ap()
    out = nc.dram_tensor("out", [C, HALF], dt.bfloat16, kind="ExternalOutput").ap()

    with tile.TileContext(nc) as tc:
        with ExitStack() as ctx:
            consts = ctx.enter_context(tc.tile_pool(name="consts", bufs=1))
            xpool = ctx.enter_context(tc.tile_pool(name="xpool", bufs=3))
            xtpool = ctx.enter_context(tc.tile_pool(name="xtpool", bufs=1))
            mpool = ctx.enter_context(tc.tile_pool(name="mpool", bufs=4))
            wapool = ctx.enter_context(tc.tile_pool(name="wapool", bufs=2))
            ypool = ctx.enter_context(tc.tile_pool(name="ypool", bufs=8))
            ztpool = ctx.enter_context(tc.tile_pool(name="ztpool", bufs=3))
            zpool = ctx.enter_context(tc.tile_pool(name="zpool", bufs=2))
            psA = ctx.enter_context(tc.tile_pool(name="psA", bufs=3, space="PSUM"))
            psB = ctx.enter_context(tc.tile_pool(name="psB", bufs=5, space="PSUM"))

            # ---- constants (w1 split per conv so layer-0 work starts early;
            # weights go on the scalar queue, x on sync, masks on gpsimd so the
            # critical head transfers run in parallel) ----
            w1_sb = consts.tile([128, 36, 512], dt.bfloat16)
            # layer 0 pieces first (yf conv first: it is the first matmul issued);
            # l1/l2 pieces are emitted later on the gpsimd queue behind l0 masks.
            for cv in (2, 1, 0):
                i0 = cv * 4
                nc.scalar.dma_start(out=w1_sb[:, i0 : i0 + 4, :], in_=w1[:, i0 : i0 + 4, :])
            bsum_sb = consts.tile([1, 3 * 512], dt.bfloat16)
            nc.gpsimd.dma_start(out=bsum_sb[:], in_=bsum)
            ba_sb = consts.tile([1, 3 * 512], dt.bfloat16)
            nc.gpsimd.dma_start(out=ba_sb[:], in_=ba_r)
            em_sb = consts.tile([128, 2 * NCHUNK], dt.float32)
            nc.gpsimd.dma_start(
                out=em_sb[:],
                in_=bass.AP(tensor=em.tensor, offset=0, ap=[[0, 128], [1, 2 * NCHUNK]]),
            )
            ones128 = consts.tile([1, 128], dt.bfloat16)
            nc.vector.memset(ones128[:], 1.0)
            ones512 = consts.tile([1, 512], dt.bfloat16)
            nc.vector.memset(ones512[:], 1.0)

            for ck in range(NCHUNK):
                cb = ck * TC

                x_cur = xpool.tile([128, 4, E], dt.bfloat16, tag="x")
                xt = xtpool.tile([128, 4, E], dt.bfloat16, tag="xt")
                npiece = 8 if ck == 0 else 4
                pw = E // npiece
                for q in range(npiece):
                    c0 = q * pw
                    nc.sync.dma_start(
                        out=x_cur[:, :, c0 : c0 + pw],
                        in_=xs[:, cb + c0 : cb + c0 + pw].rearrange("(a p) t -> p a t", p=128),
                    )
                    nc.scalar.activation(
                        out=xt[:, :, c0 : c0 + pw], in_=x_cur[:, :, c0 : c0 + pw],
                        func=mybir.ActivationFunctionType.Prelu, alpha=SLOPE,
                    )

                for l in range(3):
                    masks = {}
                    for i, nm in enumerate(("P_diag", "F_diag", "F_sup", "P_sub")):
                        mk = mpool.tile([128, E], dt.bfloat16, tag="mask")
                        r0 = (l * 4 + (0, 2, 3, 1)[i]) * 128
                        nc.gpsimd.dma_start(out=mk[:, 0 : E // 2], in_=mks[r0 : r0 + 128, cb : cb + E // 2])
                        nc.gpsimd.dma_start(out=mk[:, E // 2 : E], in_=mks[r0 : r0 + 128, cb + E // 2 : cb + E])
                        masks[nm] = mk
                    wa_sb = wapool.tile([128, 12, 512], dt.bfloat16, tag="wa")
                    nc.sync.dma_start(out=wa_sb[:], in_=wa[:, l * 12 : (l + 1) * 12, :])
                    if ck == 0 and l == 0:
                        # deferred: layer-1/2 1x1 weights, behind layer-0 masks
                        for ll in (1, 2):
                            for cv in (2, 1, 0):
                                i0 = (ll * 3 + cv) * 4
                                nc.gpsimd.dma_start(out=w1_sb[:, i0 : i0 + 4, :], in_=w1[:, i0 : i0 + 4, :])

                    ycT = [None] * NT
                    ypS = [None] * NT
                    yfS = [None] * NT
                    zTs = [None] * NT

                    def conv_step(j):
                        yc = psA.tile([128, 512], dt.float32, tag="ycT")
                        yp = psB.tile([128, 512], dt.float32, tag="ps")
                        yf = psB.tile([128, 512], dt.float32, tag="ps")
                        for a in range(4):
                            lhs = xt[:, a, 128 * j : 128 * (j + 1)]
                            st = a == 0
                            nc.tensor.matmul(yf[:], lhs, w1_sb[:, (l * 3 + 2) * 4 + a, :], start=st, stop=a == 3)
                            nc.tensor.matmul(yp[:], lhs, w1_sb[:, (l * 3 + 1) * 4 + a, :], start=st, stop=a == 3)
                            nc.tensor.matmul(yc[:], lhs, w1_sb[:, (l * 3 + 0) * 4 + a, :], start=st, stop=False)
                        if with_bias:
                            nc.tensor.matmul(yc[:], ones128[:], bsum_sb[:, l * 512 : (l + 1) * 512], start=False, stop=False)
                        ycT[j] = yc
                        yp_s = ypool.tile([128, 512], dt.bfloat16, tag="yps")
                        nc.vector.tensor_copy(out=yp_s[:], in_=yp[:])
                        yf_s = ypool.tile([128, 512], dt.bfloat16, tag="yps")
                        nc.scalar.activation(out=yf_s[:], in_=yf[:], func=mybir.ActivationFunctionType.Copy)
                        ypS[j] = yp_s
                        yfS[j] = yf_s

                    def gather_step(j):
                        tj = slice(128 * j, 128 * (j + 1))
                        # F_sup last: its rhs yfS[j+1] is the freshest copy, so the
                        # preceding three matmuls cover the PSUM->SBUF copy latency.
                        if j > 0:
                            nc.tensor.matmul(ycT[j][:], masks["P_sub"][:, tj], ypS[j - 1][:], start=False, stop=False)
                        nc.tensor.matmul(ycT[j][:], masks["P_diag"][:, tj], ypS[j][:], start=False, stop=False)
                        nc.tensor.matmul(ycT[j][:], masks["F_diag"][:, tj], yfS[j][:], start=False, stop=j == NT - 1)
                        if j < NT - 1:
                            nc.tensor.matmul(ycT[j][:], masks["F_sup"][:, tj], yfS[j + 1][:], start=False, stop=True)
                        zt = ztpool.tile([128, 512], dt.bfloat16, tag="zt")
                        nc.scalar.activation(out=zt[:], in_=ycT[j][:], func=mybir.ActivationFunctionType.Prelu, alpha=SLOPE)
                        zTs[j] = zt

                    z = zpool.tile([128, 4, E], dt.bfloat16, tag="z")

                    def transpose_step(j):
                        eng = nc.sync if j % 2 == 0 else nc.scalar
                        eng.dma_start_transpose(out=z[:, :, 128 * j : 128 * (j + 1)], in_=zTs[j][:])
                        if j == 0:
                            for m in range(4):
                                nc.vector.tensor_scalar(
                                    out=z[:, m, H - 1 : H], in0=z[:, m, H - 1 : H],
                                    scalar1=em_sb[:, 2 * ck : 2 * ck + 1], scalar2=None,
                                    op0=mybir.AluOpType.mult,
                                )
                        if j == NT - 1:
                            for m in range(4):
                                nc.vector.tensor_scalar(
                                    out=z[:, m, H + TC : H + TC + 1], in0=z[:, m, H + TC : H + TC + 1],
                                    scalar1=em_sb[:, 2 * ck + 1 : 2 * ck + 2], scalar2=None,
                                    op0=mybir.AluOpType.mult,
                                )

                    for j in range(NT):
                        conv_step(j)
                        if j >= 1:
                            gather_step(j - 1)
                            transpose_step(j - 1)
                    gather_step(NT - 1)
                    transpose_step(NT - 1)

                    # ---- k=3 conv + residual, window-outer so early windows
                    # overlap the tail of the conv/gather/transpose phase ----
                    x_next = xpool.tile([128, 4, E], dt.bfloat16, tag="x")
                    last = l == 2
                    if not last:
                        xt_n = xtpool.tile([128, 4, E], dt.bfloat16, tag="xt")
                        for m in range(4):
                            nc.vector.memset(xt_n[:, m, 0:1], 0.0)
                            nc.vector.memset(xt_n[:, m, E - 1 : E], 0.0)
                    for wi, (w0, wn) in enumerate(K3WIN):
                        for m in range(4):
                            pk = psB.tile([128, 512], dt.float32, tag="ps")
                            for ki, (tau, a) in enumerate(TAUA):
                                lhs = wa_sb[:, tau * 4 + a, 128 * m : 128 * (m + 1)]
                                rhs = z[:, a, w0 + tau - 1 : w0 + tau - 1 + wn]
                                nc.tensor.matmul(pk[:, 0:wn], lhs, rhs, start=ki == 0,
                                                 stop=(ki == 11) and not with_bias)
                            if with_bias:
                                nc.tensor.matmul(
                                    pk[:, 0:wn], ba_sb[:, l * 512 + 128 * m : l * 512 + 128 * (m + 1)],
                                    ones512[:, 0:wn], start=False, stop=True,
                                )
                            nc.vector.tensor_tensor(
                                out=x_next[:, m, w0 : w0 + wn], in0=pk[:, 0:wn],
                                in1=x_cur[:, m, w0 : w0 + wn], op=mybir.AluOpType.add,
                            )
                            if not last:
                                nc.scalar.activation(
                                    out=xt_n[:, m, w0 : w0 + wn], in_=x_next[:, m, w0 : w0 + wn],
                                    func=mybir.ActivationFunctionType.Prelu, alpha=SLOPE,
                                )
                        if last:
                            lo, hi = max(w0, H), min(w0 + wn, H + TC)
                            if hi > lo:
                                nc.sync.dma_start(
                                    out=out[:, cb + lo - H : cb + hi - H].rearrange("(a p) t -> p a t", p=128),
                                    in_=x_next[:, :, lo:hi],
                                )
                    x_cur = x_next
                    if not last:
                        xt = xt_n

    nc.compile()
    return nc


def _host_inputs(x, d, Wc, bc, Wp, bp, Wf, bf, Wa, ba):
    x = np.asarray(x, dtype=F32)
    d = np.asarray(d, dtype=F32)
    Wc, Wp, Wf = (np.asarray(w, dtype=F32) for w in (Wc, Wp, Wf))
    Wa = np.asarray(Wa, dtype=F32)
    bc, bp, bf, ba = (np.asarray(v, dtype=F32) for v in (bc, bp, bf, ba))

    w1 = np.empty((128, 36, 512), dtype=BF16)
    wa = np.empty((128, 36, 512), dtype=BF16)
    for l in range(3):
        for cv, W in enumerate((Wc, Wp, Wf)):
            wt = W[l].T.astype(BF16)
            for a in range(4):
                w1[:, (l * 3 + cv) * 4 + a, :] = wt[a * 128 : (a + 1) * 128, :]
        for tau in range(3):
            wt = Wa[l][:, :, tau].T.astype(BF16)
            for a in range(4):
                wa[:, (l * 3 + tau) * 4 + a, :] = wt[a * 128 : (a + 1) * 128, :]
    bsum = (bc + bp + bf).reshape(1, -1).astype(BF16)
    ba_r = ba.reshape(1, -1).astype(BF16)

    p_ar = np.arange(128, dtype=np.int64)[:, None]
    tilebase = 128 * (np.arange(SL, dtype=np.int64) // 128)[None, :]

    in_maps = []
    for core in range(NCORES):
        b, h = core // 2, core % 2
        g0 = h * HALF
        lo = g0 - H
        xsl = np.zeros((C, SL), dtype=BF16)
        dsl = np.zeros((1, SL), dtype=F32)
        s0 = max(0, lo)
        s1 = min(T, g0 + HALF + H)
        xsl[:, s0 - lo : s1 - lo] = x[b, :, s0:s1].astype(BF16)
        dsl[:, s0 - lo : s1 - lo] = d[b, :, s0:s1]

        tg = (np.arange(SL, dtype=np.float64) + lo).astype(F32)
        cl = float(max(0, lo))
        chq = float(min(T - 1, g0 + HALF + H - 1))
        mks = np.zeros((12 * 128, SL), dtype=BF16)
        for l in range(3):
            dil = np.float32(DILATIONS[l])
            dila = (dsl[0] * dil).astype(F32)
            for gi, sgn in ((0, np.float32(-1.0)), (1, np.float32(1.0))):
                u = (tg + sgn * dila).astype(F32)
                idxg = np.clip(np.round(u), cl, chq).astype(np.int64)
                rel = (idxg - lo)[None, :] - tilebase
                diag = (rel == p_ar).astype(BF16)
                off = (rel == (p_ar - 128)).astype(BF16) if gi == 0 else (rel == (p_ar + 128)).astype(BF16)
                i_diag = l * 4 + (0 if gi == 0 else 2)
                i_off = l * 4 + (1 if gi == 0 else 3)
                mks[i_diag * 128 : (i_diag + 1) * 128, :] = diag
                mks[i_off * 128 : (i_off + 1) * 128, :] = off

        em = np.ones((1, 2 * NCHUNK), dtype=F32)
        if h == 0:
            em[0, 0] = 0.0
        if h == 1:
            em[0, 2 * NCHUNK - 1] = 0.0
        in_maps.append(
            dict(xs=xsl, mks=mks, em=em, w1=w1, wa=wa, bsum=bsum, ba_r=ba_r)
        )
    return in_maps


_NC_BIAS = None


def kernel(**inputs):
    global _NC, _NC_BIAS
    wb = any(np.any(np.asarray(inputs[k])) for k in ("bc", "bp", "bf", "ba"))
    if _NC is None or _NC_BIAS != wb:
        _NC = _build(with_bias=wb)
        _NC_BIAS = wb
    in_maps = _host_inputs(**inputs)
    res = run_bass_kernel_spmd(_NC, in_maps, core_ids=list(range(NCORES)), trace=False)
    out = np.empty((B, C, T), dtype=F32)
    for core in range(NCORES):
        b, h = core // 2, core % 2
        out[b, :, h * HALF : (h + 1) * HALF] = np.asarray(res.results[core]["out"]).astype(F32)
    return out


# revision 26
# speedup vs baseline: 1.2830x; 1.0037x over previous
import sys, os
sys.path.insert(0, "/opt/trn_rl_repo")
import numpy as np
import ml_dtypes
from contextlib import ExitStack

import concourse.bass as bass
import concourse.tile as tile
from concourse import bacc, mybir
from concourse.bass_utils import run_bass_kernel_spmd

BF16 = ml_dtypes.bfloat16
F32 = np.float32

B, C, T = 4, 512, 16384
DILATIONS = (1, 2, 4)
SLOPE = 0.1
NCORES = 8
HALF = T // 2            # 8192 per core
H = 64                   # halo per side (max reach used: 3+1 + 6+1 + 12+1 = 24)
SL = HALF + 2 * H        # 8320 slice length
TC = 2048                # chunk kept size
E = TC + 2 * H           # 2176 chunk buffer extent
NCHUNK = HALF // TC      # 4
NT = E // 128            # 17 t-tiles per chunk buffer
K3WIN = [(1, 435), (436, 435), (871, 435), (1306, 435), (1741, 434)]
TAUA = [(t_, a_) for t_ in range(3) for a_ in range(4)]
EQ = E // 4              # 544, quarter split for x DMA / prelu pipelining

_NC = None


def _build(with_bias=True):
    nc = bacc.Bacc("TRN2", target_bir_lowering=False, debug=False, num_devices=NCORES)
    dt = mybir.dt

    xs = nc.dram_tensor("xs", [C, SL], dt.bfloat16, kind="ExternalInput").ap()
    mks = nc.dram_tensor("mks", [12 * 128, SL], dt.bfloat16, kind="ExternalInput").ap()
    em = nc.dram_tensor("em", [1, 2 * NCHUNK], dt.float32, kind="ExternalInput").ap()
    w1 = nc.dram_tensor("w1", [128, 36, 512], dt.bfloat16, kind="ExternalInput").ap()
    wa = nc.dram_tensor("wa", [128, 36, 512], dt.bfloat16, kind="ExternalInput").ap()
    bsum = nc.dram_tensor("bsum", [1, 3 * 512], dt.bfloat16, kind="ExternalInput").ap()
    ba_r = nc.dram_tensor("ba_r", [1, 3 * 512], dt.bfloat16, kind="ExternalInput").ap()
    out = nc.dram_tensor("out", [C, HALF], dt.bfloat16, kind="ExternalOutput").ap()

    with tile.TileContext(nc) as tc:
        with ExitStack() as ctx:
            consts = ctx.enter_context(tc.tile_pool(name="consts", bufs=1))
            xpool = ctx.enter_context(tc.tile_pool(name="xpool", bufs=3))
            xtpool = ctx.enter_context(tc.tile_pool(name="xtpool", bufs=1))
            mpool = ctx.enter_context(tc.tile_pool(name="mpool", bufs=4))
            wapool = ctx.enter_context(tc.tile_pool(name="wapool", bufs=2))
            ypool = ctx.enter_context(tc.tile_pool(name="ypool", bufs=8))
            ztpool = ctx.enter_context(tc.tile_pool(name="ztpool", bufs=3))
            zpool = ctx.enter_context(tc.tile_pool(name="zpool", bufs=2))
            psA = ctx.enter_context(tc.tile_pool(name="psA", bufs=3, space="PSUM"))
            psB = ctx.enter_context(tc.tile_pool(name="psB", bufs=5, space="PSUM"))

            # ---- constants (w1 split per conv so layer-0 work starts early;
            # weights go on the scalar queue, x on sync, masks on gpsimd so the
            # critical head transfers run in parallel) ----
            w1_sb = consts.tile([128, 36, 512], dt.bfloat16)
            for l in range(3):
                for cv in range(3):
                    i0 = (l * 3 + cv) * 4
                    nc.scalar.dma_start(out=w1_sb[:, i0 : i0 + 4, :], in_=w1[:, i0 : i0 + 4, :])
            bsum_sb = consts.tile([1, 3 * 512], dt.bfloat16)
            nc.gpsimd.dma_start(out=bsum_sb[:], in_=bsum)
            ba_sb = consts.tile([1, 3 * 512], dt.bfloat16)
            nc.gpsimd.dma_start(out=ba_sb[:], in_=ba_r)
            em_sb = consts.tile([128, 2 * NCHUNK], dt.float32)
            nc.gpsimd.dma_start(
                out=em_sb[:],
                in_=bass.AP(tensor=em.tensor, offset=0, ap=[[0, 128], [1, 2 * NCHUNK]]),
            )
            ones128 = consts.tile([1, 128], dt.bfloat16)
            nc.vector.memset(ones128[:], 1.0)
            ones512 = consts.tile([1, 512], dt.bfloat16)
            nc.vector.memset(ones512[:], 1.0)

            for ck in range(NCHUNK):
                cb = ck * TC

                x_cur = xpool.tile([128, 4, E], dt.bfloat16, tag="x")
                xt = xtpool.tile([128, 4, E], dt.bfloat16, tag="xt")
                for q in range(4):
                    c0 = q * EQ
                    nc.sync.dma_start(
                        out=x_cur[:, :, c0 : c0 + EQ],
                        in_=xs[:, cb + c0 : cb + c0 + EQ].rearrange("(a p) t -> p a t", p=128),
                    )
                    nc.scalar.activation(
                        out=xt[:, :, c0 : c0 + EQ], in_=x_cur[:, :, c0 : c0 + EQ],
                        func=mybir.ActivationFunctionType.Prelu, alpha=SLOPE,
                    )

                for l in range(3):
                    masks = {}
                    for i, nm in enumerate(("P_diag", "F_diag", "F_sup", "P_sub")):
                        mk = mpool.tile([128, E], dt.bfloat16, tag="mask")
                        r0 = (l * 4 + (0, 2, 3, 1)[i]) * 128
                        nc.gpsimd.dma_start(out=mk[:, 0 : E // 2], in_=mks[r0 : r0 + 128, cb : cb + E // 2])
                        nc.gpsimd.dma_start(out=mk[:, E // 2 : E], in_=mks[r0 : r0 + 128, cb + E // 2 : cb + E])
                        masks[nm] = mk
                    wa_sb = wapool.tile([128, 12, 512], dt.bfloat16, tag="wa")
                    nc.sync.dma_start(out=wa_sb[:], in_=wa[:, l * 12 : (l + 1) * 12, :])

                    ycT = [None] * NT
                    ypS = [None] * NT
                    yfS = [None] * NT
                    zTs = [None] * NT

                    def conv_step(j):
                        yc = psA.tile([128, 512], dt.float32, tag="ycT")
                        yp = psB.tile([128, 512], dt.float32, tag="ps")
                        yf = psB.tile([128, 512], dt.float32, tag="ps")
                        for a in range(4):
                            lhs = xt[:, a, 128 * j : 128 * (j + 1)]
                            st = a == 0
                            nc.tensor.matmul(yf[:], lhs, w1_sb[:, (l * 3 + 2) * 4 + a, :], start=st, stop=a == 3)
                            nc.tensor.matmul(yp[:], lhs, w1_sb[:, (l * 3 + 1) * 4 + a, :], start=st, stop=a == 3)
                            nc.tensor.matmul(yc[:], lhs, w1_sb[:, (l * 3 + 0) * 4 + a, :], start=st, stop=False)
                        if with_bias:
                            nc.tensor.matmul(yc[:], ones128[:], bsum_sb[:, l * 512 : (l + 1) * 512], start=False, stop=False)
                        ycT[j] = yc
                        yp_s = ypool.tile([128, 512], dt.bfloat16, tag="yps")
                        nc.vector.tensor_copy(out=yp_s[:], in_=yp[:])
                        yf_s = ypool.tile([128, 512], dt.bfloat16, tag="yps")
                        nc.scalar.activation(out=yf_s[:], in_=yf[:], func=mybir.ActivationFunctionType.Copy)
                        ypS[j] = yp_s
                        yfS[j] = yf_s

                    def gather_step(j):
                        tj = slice(128 * j, 128 * (j + 1))
                        # F_sup last: its rhs yfS[j+1] is the freshest copy, so the
                        # preceding three matmuls cover the PSUM->SBUF copy latency.
                        if j > 0:
                            nc.tensor.matmul(ycT[j][:], masks["P_sub"][:, tj], ypS[j - 1][:], start=False, stop=False)
                        nc.tensor.matmul(ycT[j][:], masks["P_diag"][:, tj], ypS[j][:], start=False, stop=False)
                        nc.tensor.matmul(ycT[j][:], masks["F_diag"][:, tj], yfS[j][:], start=False, stop=j == NT - 1)
                        if j < NT - 1:
                            nc.tensor.matmul(ycT[j][:], masks["F_sup"][:, tj], yfS[j + 1][:], start=False, stop=True)
                        zt = ztpool.tile([128, 512], dt.bfloat16, tag="zt")
                        nc.scalar.activation(out=zt[:], in_=ycT[j][:], func=mybir.ActivationFunctionType.Prelu, alpha=SLOPE)
                        zTs[j] = zt

                    z = zpool.tile([128, 4, E], dt.bfloat16, tag="z")

                    def transpose_step(j):
                        eng = nc.sync if j % 2 == 0 else nc.scalar
                        eng.dma_start_transpose(out=z[:, :, 128 * j : 128 * (j + 1)], in_=zTs[j][:])
                        if j == 0:
                            for m in range(4):
                                nc.vector.tensor_scalar(
                                    out=z[:, m, H - 1 : H], in0=z[:, m, H - 1 : H],
                                    scalar1=em_sb[:, 2 * ck : 2 * ck + 1], scalar2=None,
                                    op0=mybir.AluOpType.mult,
                                )
                        if j == NT - 1:
                            for m in range(4):
                                nc.vector.tensor_scalar(
                                    out=z[:, m, H + TC : H + TC + 1], in0=z[:, m, H + TC : H + TC + 1],
                                    scalar1=em_sb[:, 2 * ck + 1 : 2 * ck + 2], scalar2=None,
                                    op0=mybir.AluOpType.mult,
                                )

                    for j in range(NT):
                        conv_step(j)
                        if j >= 1:
                            gather_step(j - 1)
                            transpose_step(j - 1)
                    gather_step(NT - 1)
                    transpose_step(NT - 1)

                    # ---- k=3 conv + residual, window-outer so early windows
                    # overlap the tail of the conv/gather/transpose phase ----
                    x_next = xpool.tile([128, 4, E], dt.bfloat16, tag="x")
                    last = l == 2
                    if not last:
                        xt_n = xtpool.tile([128, 4, E], dt.bfloat16, tag="xt")
                        for m in range(4):
                            nc.vector.memset(xt_n[:, m, 0:1], 0.0)
                            nc.vector.memset(xt_n[:, m, E - 1 : E], 0.0)
                    for wi, (w0, wn) in enumerate(K3WIN):
                        for m in range(4):
                            pk = psB.tile([128, 512], dt.float32, tag="ps")
                            for ki, (tau, a) in enumerate(TAUA):
                                lhs = wa_sb[:, tau * 4 + a, 128 * m : 128 * (m + 1)]
                                rhs = z[:, a, w0 + tau - 1 : w0 + tau - 1 + wn]
                                nc.tensor.matmul(pk[:, 0:wn], lhs, rhs, start=ki == 0,
                                                 stop=(ki == 11) and not with_bias)
                            if with_bias:
                                nc.tensor.matmul(
                                    pk[:, 0:wn], ba_sb[:, l * 512 + 128 * m : l * 512 + 128 * (m + 1)],
                                    ones512[:, 0:wn], start=False, stop=True,
                                )
                            nc.vector.tensor_tensor(
                                out=x_next[:, m, w0 : w0 + wn], in0=pk[:, 0:wn],
                                in1=x_cur[:, m, w0 : w0 + wn], op=mybir.AluOpType.add,
                            )
                            if not last:
                                nc.scalar.activation(
                                    out=xt_n[:, m, w0 : w0 + wn], in_=x_next[:, m, w0 : w0 + wn],
                                    func=mybir.ActivationFunctionType.Prelu, alpha=SLOPE,
                                )
                        if last:
                            lo, hi = max(w0, H), min(w0 + wn, H + TC)
                            if hi > lo:
                                nc.sync.dma_start(
                                    out=out[:, cb + lo - H : cb + hi - H].rearrange("(a p) t -> p a t", p=128),
                                    in_=x_next[:, :, lo:hi],
                                )
                    x_cur = x_next
                    if not last:
                        xt = xt_n

    nc.compile()
    return nc


def _host_inputs(x, d, Wc, bc, Wp, bp, Wf, bf, Wa, ba):
    x = np.asarray(x, dtype=F32)
    d = np.asarray(d, dtype=F32)
    Wc, Wp, Wf = (np.asarray(w, dtype=F32) for w in (Wc, Wp, Wf))
    Wa = np.asarray(Wa, dtype=F32)
    bc, bp, bf, ba = (np.asarray(v, dtype=F32) for v in (bc, bp, bf, ba))

    w1 = np.empty((128, 36, 512), dtype=BF16)
    wa = np.empty((128, 36, 512), dtype=BF16)
    for l in range(3):
        for cv, W in enumerate((Wc, Wp, Wf)):
            wt = W[l].T.astype(BF16)
            for a in range(4):
                w1[:, (l * 3 + cv) * 4 + a, :] = wt[a * 128 : (a + 1) * 128, :]
        for tau in range(3):
            wt = Wa[l][:, :, tau].T.astype(BF16)
            for a in range(4):
                wa[:, (l * 3 + tau) * 4 + a, :] = wt[a * 128 : (a + 1) * 128, :]
    bsum = (bc + bp + bf).reshape(1, -1).astype(BF16)
    ba_r = ba.reshape(1, -1).astype(BF16)

    p_ar = np.arange(128, dtype=np.int64)[:, None]
    tilebase = 128 * (np.arange(SL, dtype=np.int64) // 128)[None, :]

    in_maps = []
    for core in range(NCORES):
        b, h = core // 2, core % 2
        g0 = h * HALF
        lo = g0 - H
        xsl = np.zeros((C, SL), dtype=BF16)
        dsl = np.zeros((1, SL), dtype=F32)
        s0 = max(0, lo)
        s1 = min(T, g0 + HALF + H)
        xsl[:, s0 - lo : s1 - lo] = x[b, :, s0:s1].astype(BF16)
        dsl[:, s0 - lo : s1 - lo] = d[b, :, s0:s1]

        tg = (np.arange(SL, dtype=np.float64) + lo).astype(F32)
        cl = float(max(0, lo))
        chq = float(min(T - 1, g0 + HALF + H - 1))
        mks = np.zeros((12 * 128, SL), dtype=BF16)
        for l in range(3):
            dil = np.float32(DILATIONS[l])
            dila = (dsl[0] * dil).astype(F32)
            for gi, sgn in ((0, np.float32(-1.0)), (1, np.float32(1.0))):
                u = (tg + sgn * dila).astype(F32)
                idxg = np.clip(np.round(u), cl, chq).astype(np.int64)
                rel = (idxg - lo)[None, :] - tilebase
                diag = (rel == p_ar).astype(BF16)
                off = (rel == (p_ar - 128)).astype(BF16) if gi == 0 else (rel == (p_ar + 128)).astype(BF16)
                i_diag = l * 4 + (0 if gi == 0 else 2)
                i_off = l * 4 + (1 if gi == 0 else 3)
                mks[i_diag * 128 : (i_diag + 1) * 128, :] = diag
                mks[i_off * 128 : (i_off + 1) * 128, :] = off

        em = np.ones((1, 2 * NCHUNK), dtype=F32)
        if h == 0:
            em[0, 0] = 0.0
        if h == 1:
            em[0, 2 * NCHUNK - 1] = 0.0
        in_maps.append(
            dict(xs=xsl, mks=mks, em=em, w1=w1, wa=wa, bsum=bsum, ba_r=ba_r)
        )
    return in_maps


_NC_BIAS = None


def kernel(**inputs):
    global _NC, _NC_BIAS
    wb = any(np.any(np.asarray(inputs[k])) for k in ("bc", "bp", "bf", "ba"))
    if _NC is None or _NC_BIAS != wb:
        _NC = _build(with_bias=wb)
        _NC_BIAS = wb
    in_maps = _host_inputs(**inputs)
    res = run_bass_kernel_spmd(_NC, in_maps, core_ids=list(range(NCORES)), trace=False)
    out = np.empty((B, C, T), dtype=F32)
    for core in range(NCORES):
        b, h = core // 2, core % 2
        out[b, :, h * HALF : (h + 1) * HALF] = np.asarray(res.results[core]["out"]).astype(F32)
    return out
